# revision 18
# baseline (speedup 1.0000x reference)
"""ConvCNP forward kernel for Trainium2 (8 NeuronCores, data-parallel over batch).

Math (per batch b):
  w_x[n,g]   = exp(-(t_g - xc_n)^2 / (2 ls_x^2))
  h[c,g]     = sum_n phi[c,n] * w_x[n,g],  phi = [1, y_c]
  h          = [h0, h1/(h0+1e-8)]
  y          = SimpleCNN(h)                      (k=5 SAME convs, 2-16-32-16-2)
  y_grid     = [y0, softplus(y1)]
  w_rho[t,g] = exp(-(xt_t - t_g)^2 / (2 ls_rho^2))
  y_pred[c,t]= sum_g w_rho[t,g] * y_grid[c,g]
  outputs mu = y_pred[0], sigma = y_pred[1]; loss computed on host from mu/sigma.

Device-side trick: exp(-(t-x)^2*a) = C(t) * exp(s0(x)*t + s1(x)) with
  s0 = 2*a*x, s1 = -a*x^2, C = exp(-a*t^2)   (C, s0, s1 precomputed on host),
so each kernel-weight tile costs one DVE tensor_scalar (fma) + one ACT Exp,
and the per-column factor C folds into a single post-matmul multiply.
"""

import os
import numpy as np

G = 1000
NCTX = 2048
NT = 2048
B = 32
NCORES = 8
BLOC = B // NCORES          # 4 batches per core
KT = NCTX // 128            # 16 context k-tiles
GT = (G + 127) // 128       # 8 grid g-tiles (last has 104 valid rows)
GPAD = GT * 128
LOG_2PI = float(np.log(2.0 * np.pi))
CONV = [(2, 16), (16, 32), (32, 16), (16, 2)]   # (Cin, Cout) per layer

_cache = {}


def _build():
    """Build + compile the per-core Bass/Tile kernel (shapes only; no data baked)."""
    from contextlib import ExitStack

    import concourse.bacc as bacc
    import concourse.tile as tile
    from concourse import mybir

    f32 = mybir.dt.float32
    AF = mybir.ActivationFunctionType
    OP = mybir.AluOpType

    nc = bacc.Bacc("TRN2", target_bir_lowering=False, debug=False,
                   num_devices=NCORES)

    # ---- DRAM I/O -----------------------------------------------------------
    tg_d = nc.dram_tensor("tg", [G], f32, kind="ExternalInput").ap()
    xs_d = nc.dram_tensor("xs", [128, BLOC * KT * 2], f32, kind="ExternalInput").ap()
    cg2_d = nc.dram_tensor("cg2", [2, G], f32, kind="ExternalInput").ap()
    yc_d = nc.dram_tensor("yc", [BLOC, NCTX], f32, kind="ExternalInput").ap()
    ts_d = nc.dram_tensor("ts", [128, GT * 2], f32, kind="ExternalInput").ap()
    xt_d = nc.dram_tensor("xt", [BLOC, NT], f32, kind="ExternalInput").ap()
    c2_d = nc.dram_tensor("c2", [BLOC, 2, NT], f32, kind="ExternalInput").ap()
    wc_d = nc.dram_tensor("wc", [32, 330], f32, kind="ExternalInput").ap()
    bc_d = nc.dram_tensor("bc", [32, 5], f32, kind="ExternalInput").ap()
    id_d = nc.dram_tensor("ident", [128, 128], f32, kind="ExternalInput").ap()
    mu_d = nc.dram_tensor("mu", [BLOC, NT], f32, kind="ExternalOutput").ap()
    sg_d = nc.dram_tensor("sg", [BLOC, NT], f32, kind="ExternalOutput").ap()
    dbg = bool(int(os.environ.get("BASS_KERNEL_DEBUG", "0")))
    if dbg:
        dbg_d = nc.dram_tensor("dbgyg", [BLOC, 2, G], f32,
                               kind="ExternalOutput").ap()

    woff = []
    off = 0
    for (_ci, co) in CONV:
        woff.append(off)
        off += 5 * co

    with tile.TileContext(nc) as tc, ExitStack() as ctx:
        singles = ctx.enter_context(tc.tile_pool(name="singles", bufs=1))
        p1p = ctx.enter_context(tc.tile_pool(name="p1p", bufs=2))
        e1p = ctx.enter_context(tc.tile_pool(name="e1p", bufs=2))
        p2p = ctx.enter_context(tc.tile_pool(name="p2p", bufs=2))
        e2p = ctx.enter_context(tc.tile_pool(name="e2p", bufs=2))
        xtbp = ctx.enter_context(tc.tile_pool(name="xtbp", bufs=2))
        cinp = ctx.enter_context(tc.tile_pool(name="cinp", bufs=2))
        ygp = ctx.enter_context(tc.tile_pool(name="ygp", bufs=2))
        ygtp = ctx.enter_context(tc.tile_pool(name="ygtp", bufs=2))
        ysp = ctx.enter_context(tc.tile_pool(name="ysp", bufs=2))
        c2sp = ctx.enter_context(tc.tile_pool(name="c2sp", bufs=2))
        smalls = ctx.enter_context(tc.tile_pool(name="smalls", bufs=2))
        psS = ctx.enter_context(tc.tile_pool(name="psS", bufs=2, space="PSUM"))
        psY = ctx.enter_context(tc.tile_pool(name="psY", bufs=1, space="PSUM"))

        # ---- constants into SBUF -------------------------------------------
        tb = singles.tile([128, G], f32)                       # t broadcast on parts
        nc.sync.dma_start(out=tb, in_=tg_d.partition_broadcast(128))
        xs_sb = singles.tile([128, BLOC * KT * 2], f32)
        nc.sync.dma_start(out=xs_sb, in_=xs_d)
        cg2_sb = singles.tile([2, G], f32)
        nc.sync.dma_start(out=cg2_sb, in_=cg2_d)
        ts_sb = singles.tile([128, GT * 2], f32)
        nc.sync.dma_start(out=ts_sb, in_=ts_d)
        wc_sb = singles.tile([32, 330], f32)
        nc.sync.dma_start(out=wc_sb, in_=wc_d)
        bc_sb = singles.tile([32, 5], f32)
        nc.sync.dma_start(out=bc_sb, in_=bc_d)
        id_sb = singles.tile([128, 128], f32)
        nc.sync.dma_start(out=id_sb, in_=id_d)
        eps_sb = singles.tile([1, 1], f32)
        nc.vector.memset(eps_sb, 1e-8)

        cg2_v = cg2_sb.rearrange("p (c x) -> p c x", x=500)    # [2,2,500]

        for b in range(BLOC):
            # ---- prefetch per-batch data ----------------------------------
            xtb = xtbp.tile([128, NT], f32)
            nc.sync.dma_start(out=xtb, in_=xt_d[b].partition_broadcast(128))
            c2s = c2sp.tile([2, NT], f32)
            nc.sync.dma_start(out=c2s, in_=c2_d[b])
            ych = smalls.tile([16, 128], f32, tag="ych")
            nc.sync.dma_start(out=ych, in_=yc_d[b].rearrange("(i p) -> i p", i=16))

            # phi lhsT: [128, 16 kt, 2] with [:, :, 0] = 1, [:, :, 1] = y_c
            tp = psS.tile([128, 16], f32, tag="ps")
            nc.tensor.transpose(tp, ych, id_sb[0:16, 0:16])
            phi = smalls.tile([128, 32], f32, tag="phi")
            phi3 = phi.rearrange("p (i c) -> p i c", c=2)
            nc.vector.memset(phi3[:, :, 0], 1.0)
            nc.vector.tensor_copy(phi3[:, :, 1], tp)

            # ---- stage A: context -> grid -----------------------------------
            hps = psS.tile([2, 1024], f32, tag="ps")           # h accum (2 banks)
            for q in range(KT // 4):
                p1 = p1p.tile([128, 4000], f32)
                for j in range(4):
                    kt = q * 4 + j
                    col = (b * KT + kt) * 2
                    nc.vector.tensor_scalar(
                        out=p1[:, j * 1000:(j + 1) * 1000], in0=tb,
                        scalar1=xs_sb[:, col:col + 1],
                        scalar2=xs_sb[:, col + 1:col + 2],
                        op0=OP.mult, op1=OP.add)
                e1 = e1p.tile([128, 4000], f32)
                nc.scalar.activation(e1, p1, AF.Exp)
                for j in range(4):
                    kt = q * 4 + j
                    for h in range(2):
                        nc.tensor.matmul(
                            hps[0:2, h * 512:h * 512 + 500],
                            lhsT=phi3[:, kt, :],
                            rhs=e1[:, j * 1000 + h * 500: j * 1000 + h * 500 + 500],
                            start=(kt == 0), stop=(kt == KT - 1))

            # ---- stage B: density normalization -----------------------------
            cin = cinp.tile([32, 1004], f32, tag="cin")
            nc.gpsimd.memset(cin[0:2, 0:2], 0.0)
            nc.gpsimd.memset(cin[0:2, 1002:1004], 0.0)
            hps_v = hps.rearrange("p (c x) -> p c x", x=512)[:, :, 0:500]
            cin_v = cin[0:2, 2:1002].rearrange("p (c x) -> p c x", x=500)
            nc.vector.tensor_tensor(out=cin_v, in0=hps_v, in1=cg2_v, op=OP.mult)
            lg = smalls.tile([1, G], f32, tag="lgrc")
            nc.scalar.activation(lg, cin[0:1, 2:1002], AF.Ln, bias=eps_sb)
            rc = smalls.tile([1, G], f32, tag="lgrc")
            nc.scalar.activation(rc, lg, AF.Exp, scale=-1.0)   # 1/(h0+eps)
            # compute-engine partition offsets must be 32-aligned, so build a
            # 2-row multiplier [1; rc] (row 1 written via DMA) and scale both
            # channels at once
            rc2 = smalls.tile([2, G], f32, tag="rc2")
            nc.gpsimd.memset(rc2[0:1, :], 1.0)
            nc.sync.dma_start(out=rc2[1:2, :], in_=rc)
            nc.vector.tensor_tensor(out=cin[0:2, 2:1002], in0=cin[0:2, 2:1002],
                                    in1=rc2, op=OP.mult)

            # ---- CNN --------------------------------------------------------
            cur = cin
            yg = None
            for l, (ci, co) in enumerate(CONV[:3]):
                cps = psS.tile([32, 1024], f32, tag="ps")
                for k in range(5):
                    for h in range(2):
                        nc.tensor.matmul(
                            cps[0:co, h * 512:h * 512 + 500],
                            lhsT=wc_sb[0:ci, woff[l] + k * co: woff[l] + (k + 1) * co],
                            rhs=cur[0:ci, k + h * 500: k + h * 500 + 500],
                            start=(k == 0), stop=(k == 4))
                cps_v = cps.rearrange("p (c x) -> p c x", x=512)[0:co, :, 0:500]
                nxt = cinp.tile([32, 1004], f32, tag="cin")
                nc.gpsimd.memset(nxt[0:co, 0:2], 0.0)
                nc.gpsimd.memset(nxt[0:co, 1002:1004], 0.0)
                nxt_v = nxt[0:co, 2:1002].rearrange("p (c x) -> p c x", x=500)
                nc.vector.tensor_scalar(
                    out=nxt_v, in0=cps_v,
                    scalar1=bc_sb[0:co, l:l + 1], scalar2=0.0,
                    op0=OP.add, op1=OP.max)                    # relu(x + bias)
                cur = nxt

            # layer 4 (16 -> 2): one M=1 matmul group per output channel so
            # both land at partition 0 (32-aligned partition offsets only)
            yg = ygp.tile([2, G], f32)
            cps4 = []
            for co in range(2):
                cp = psS.tile([1, 1024], f32, tag="ps")
                for k in range(5):
                    for h in range(2):
                        nc.tensor.matmul(
                            cp[0:1, h * 512:h * 512 + 500],
                            lhsT=wc_sb[0:16, woff[3] + k * 2 + co:
                                       woff[3] + k * 2 + co + 1],
                            rhs=cur[0:16, k + h * 500: k + h * 500 + 500],
                            start=(k == 0), stop=(k == 4))
                cps4.append(cp)
            mu_v = cps4[0].rearrange("p (c x) -> p c x", x=512)[0:1, :, 0:500]
            sg_v = cps4[1].rearrange("p (c x) -> p c x", x=512)[0:1, :, 0:500]
            yg_v0 = yg[0:1, :].rearrange("p (c x) -> p c x", x=500)
            nc.vector.tensor_scalar(out=yg_v0, in0=mu_v,
                                    scalar1=bc_sb[0:1, 3:4],
                                    scalar2=None, op0=OP.add)  # mu_grid
            # stable softplus: max(z,0) + ln(1 + exp(-|z|))
            zb = smalls.tile([1, G], f32, tag="rc2")
            nc.vector.tensor_scalar(out=zb, in0=sg_v,
                                    scalar1=bc_sb[0:1, 4:5], scalar2=None,
                                    op0=OP.add)
            na = smalls.tile([1, G], f32, tag="spx")
            nc.vector.tensor_scalar(out=na.bitcast(mybir.dt.uint32),
                                    in0=zb.bitcast(mybir.dt.uint32),
                                    scalar1=0x80000000, scalar2=None,
                                    op0=OP.bitwise_or)             # -|z|
            sp = smalls.tile([1, G], f32, tag="spx")
            nc.scalar.activation(sp, na, AF.Exp)
            sp2 = smalls.tile([1, G], f32, tag="spx")
            nc.scalar.activation(sp2, sp, AF.Ln, bias=1.0)
            rz = smalls.tile([1, G], f32, tag="lgrc")
            nc.vector.tensor_scalar(out=rz, in0=zb, scalar1=0.0, scalar2=None,
                                    op0=OP.max)
            spf = smalls.tile([1, G], f32, tag="lgrc")
            nc.vector.tensor_tensor(out=spf, in0=sp2, in1=rz, op=OP.add)
            nc.sync.dma_start(out=yg[1:2, :], in_=spf)
            if dbg:
                nc.sync.dma_start(out=dbg_d[b], in_=yg)

            # ---- transpose y_grid -> [g, c] tiles ---------------------------
            ygt = ygtp.tile([128, 2 * GT], f32)
            nc.gpsimd.memset(ygt, 0.0)
            for gt in range(GT):
                colw = min(128, G - gt * 128)
                tps = psS.tile([128, 2], f32, tag="ps")
                nc.tensor.transpose(tps[0:colw, 0:2],
                                    yg[0:2, gt * 128: gt * 128 + colw],
                                    id_sb[0:2, 0:2])
                nc.vector.tensor_copy(ygt[0:colw, gt * 2: gt * 2 + 2],
                                      tps[0:colw, 0:2])

            # ---- stage C: grid -> targets -----------------------------------
            yps = psY.tile([2, NT], f32)
            for gt in range(GT):
                p2 = p2p.tile([128, NT], f32)
                nc.vector.tensor_scalar(
                    out=p2, in0=xtb,
                    scalar1=ts_sb[:, 2 * gt: 2 * gt + 1],
                    scalar2=ts_sb[:, 2 * gt + 1: 2 * gt + 2],
                    op0=OP.mult, op1=OP.add)
                e2 = e2p.tile([128, NT], f32)
                nc.scalar.activation(e2, p2, AF.Exp)
                for c in range(4):
                    nc.tensor.matmul(
                        yps[0:2, c * 512:(c + 1) * 512],
                        lhsT=ygt[:, 2 * gt: 2 * gt + 2],
                        rhs=e2[:, c * 512:(c + 1) * 512],
                        start=(gt == 0), stop=(gt == GT - 1))
            ys = ysp.tile([2, NT], f32)
            nc.vector.tensor_tensor(out=ys, in0=yps, in1=c2s, op=OP.mult)
            nc.sync.dma_start(out=mu_d[b:b + 1, :], in_=ys[0:1, :])
            nc.sync.dma_start(out=sg_d[b:b + 1, :], in_=ys[1:2, :])

    nc.compile()
    return nc


def _host_inputs(x_context, y_context, x_target, ls_x, ls_rho,
                 W1, b1, W2, b2, W3, b3, W4, b4):
    """Per-core input dicts. All heavy math stays on device; these are O(B*N)."""
    t = np.linspace(-2.2, 2.2, G).astype(np.float32).astype(np.float64)
    inv_x = 1.0 / (2.0 * float(ls_x) ** 2)
    inv_r = 1.0 / (2.0 * float(ls_rho) ** 2)

    tg = t.astype(np.float32)
    cg = np.exp(-inv_x * t * t)
    cg2 = np.broadcast_to(cg, (2, G)).astype(np.float32).copy()

    tpad = np.zeros(GPAD, np.float64)
    tpad[:G] = t
    ts = np.zeros((128, GT, 2), np.float64)
    ts[:, :, 0] = (2.0 * inv_r * tpad).reshape(GT, 128).T
    ts[:, :, 1] = (-inv_r * tpad * tpad).reshape(GT, 128).T
    ts[:, GT - 1, :][G - (GT - 1) * 128:, :] = 0.0         # pad rows -> exp(0)=1
    ts = ts.reshape(128, GT * 2).astype(np.float32)

    wc = np.zeros((32, 330), np.float64)
    woff = 0
    for (Wl, (ci, co)) in zip((W1, W2, W3, W4), CONV):
        for k in range(5):
            wc[0:ci, woff + k * co: woff + (k + 1) * co] = np.asarray(Wl)[:, :, k].T
        woff += 5 * co
    wc = wc.astype(np.float32)
    bc = np.zeros((32, 5), np.float64)
    for l, bl in enumerate((b1, b2, b3)):
        bc[0:len(np.asarray(bl)), l] = np.asarray(bl)
    bc[0, 3] = np.asarray(b4)[0]      # L4 mu bias at partition 0
    bc[0, 4] = np.asarray(b4)[1]      # L4 sigma bias at partition 0
    bc = bc.astype(np.float32)
    ident = np.eye(128, dtype=np.float32)

    xc = np.asarray(x_context, np.float64)[:, 0, :]         # [B, NCTX]
    yc = np.asarray(y_context, np.float32)[:, 0, :]
    xt = np.asarray(x_target, np.float64)[:, 0, :]          # [B, NT]

    in_maps = []
    for c in range(NCORES):
        sl = slice(c * BLOC, (c + 1) * BLOC)
        x = xc[sl].reshape(BLOC, KT, 128)
        xs = np.stack([2.0 * inv_x * x, -inv_x * x * x], axis=-1)  # [b,kt,p,2]
        xs = xs.transpose(2, 0, 1, 3).reshape(128, BLOC * KT * 2).astype(np.float32)
        xtc = xt[sl]
        c2 = np.exp(-inv_r * xtc * xtc)[:, None, :]
        c2 = np.broadcast_to(c2, (BLOC, 2, NT)).astype(np.float32).copy()
        in_maps.append({
            "tg": tg, "xs": np.ascontiguousarray(xs), "cg2": cg2,
            "yc": np.ascontiguousarray(yc[sl]), "ts": ts,
            "xt": xtc.astype(np.float32), "c2": c2,
            "wc": wc, "bc": bc, "ident": ident,
        })
    return in_maps


last_results = None     # BassKernelResults of the most recent run (for test.py)


def kernel(x_context, y_context, x_target, y_target,
           ls_x, ls_rho, W1, b1, W2, b2, W3, b3, W4, b4):
    from concourse.bass_utils import run_bass_kernel_spmd
    global last_results

    assert x_context.shape == (B, 1, NCTX), x_context.shape
    if "nc" not in _cache:
        _cache["nc"] = _build()
    nc = _cache["nc"]

    in_maps = _host_inputs(x_context, y_context, x_target, ls_x, ls_rho,
                           W1, b1, W2, b2, W3, b3, W4, b4)
    trace = bool(int(os.environ.get("BASS_KERNEL_TRACE", "0")))
    res = run_bass_kernel_spmd(nc, in_maps, core_ids=list(range(NCORES)),
                               trace=trace)
    last_results = res

    mu = np.concatenate([r["mu"] for r in res.results], axis=0)    # [B, NT]
    sg = np.concatenate([r["sg"] for r in res.results], axis=0)

    # loss on host (float32, mirroring the reference formula)
    yt = np.asarray(y_target, np.float32)[:, 0, :]
    z = (yt - mu) / sg
    logp = -0.5 * z * z - np.log(sg) - np.float32(0.5 * LOG_2PI)
    loss = -np.sum(np.mean(logp, axis=-1))

    mu_o = mu[:, :, None].astype(np.float32)
    sg_o = sg[:, :, None].astype(np.float32)
    return mu_o, sg_o, np.float32(loss)


# revision 25
# speedup vs baseline: 1.2298x; 1.2298x over previous
"""ConvCNP forward kernel for Trainium2 (8 NeuronCores, data-parallel over batch).

Math (per batch b):
  w_x[n,g]   = exp(-(t_g - xc_n)^2 / (2 ls_x^2))
  h[c,g]     = sum_n phi[c,n] * w_x[n,g],  phi = [1, y_c]
  h          = [h0, h1/(h0+1e-8)]
  y          = SimpleCNN(h)                      (k=5 SAME convs, 2-16-32-16-2)
  y_grid     = [y0, softplus(y1)]
  w_rho[t,g] = exp(-(xt_t - t_g)^2 / (2 ls_rho^2))
  y_pred[c,t]= sum_g w_rho[t,g] * y_grid[c,g]
  outputs mu = y_pred[0], sigma = y_pred[1]; loss computed on host from mu/sigma.

Device-side trick: exp(-(t-x)^2*a) = C(t) * exp(s0(x)*t + s1(x)) with
  s0 = 2*a*x, s1 = -a*x^2, C = exp(-a*t^2)   (C, s0, s1 precomputed on host),
so each kernel-weight tile costs one fma (GPSIMD/DVE) + one ACT Exp; the
per-column factor folds into a cheap multiply (on device for stage A, on the
host for the final stage-C outputs).

Schedule: per-batch work is split into X(b) = [DMAs, context->grid interp,
target-weight exps] (wide, engine-parallel) and Y(b) = [normalize, CNN,
transpose, grid->target matmuls, output DMA] (serial chain). Emission order
X0 X1 Y0 X2 Y1 X3 Y2 Y3 keeps ACT/DVE/PE fed from other batches while a
given batch's serial chain trickles.
"""

import os
import numpy as np

G = 1000
NCTX = 2048
NT = 2048
B = 32
NCORES = 8
BLOC = B // NCORES          # 4 batches per core
KT = NCTX // 128            # 16 context k-tiles
GT = (G + 127) // 128       # 8 grid g-tiles (last has 104 valid rows)
GPAD = GT * 128
LOG_2PI = float(np.log(2.0 * np.pi))
CONV = [(2, 16), (16, 32), (32, 16), (16, 2)]   # (Cin, Cout) per layer

_cache = {}


def _build():
    """Build + compile the per-core Bass/Tile kernel (shapes only; no data baked)."""
    from contextlib import ExitStack

    import concourse.bacc as bacc
    import concourse.tile as tile
    from concourse import mybir

    f32 = mybir.dt.float32
    AF = mybir.ActivationFunctionType
    OP = mybir.AluOpType
    f32r = mybir.dt.float32r
    # fp32 matmul streams 4 cyc/row; f32r (rounded fp32) streams 1 cyc/row.
    # The verifier requires f32r matmul operands to be *produced* as f32r,
    # so every tile feeding a matmul is typed f32r end-to-end.

    nc = bacc.Bacc("TRN2", target_bir_lowering=False, debug=False,
                   num_devices=NCORES)

    # ---- DRAM I/O -----------------------------------------------------------
    tg_d = nc.dram_tensor("tg", [G], f32, kind="ExternalInput").ap()
    xs_d = nc.dram_tensor("xs", [128, BLOC * KT * 2], f32, kind="ExternalInput").ap()
    cg2_d = nc.dram_tensor("cg2", [2, G], f32, kind="ExternalInput").ap()
    ph_d = nc.dram_tensor("ph", [BLOC, 128, 2 * KT], f32r, kind="ExternalInput").ap()
    ts_d = nc.dram_tensor("ts", [128, GT * 2], f32, kind="ExternalInput").ap()
    xt_d = nc.dram_tensor("xt", [BLOC, NT], f32, kind="ExternalInput").ap()
    wc_d = nc.dram_tensor("wc", [32, 330], f32r, kind="ExternalInput").ap()
    bc_d = nc.dram_tensor("bc", [32, 5], f32, kind="ExternalInput").ap()
    id_d = nc.dram_tensor("ident", [2, 2], f32, kind="ExternalInput").ap()
    mu_d = nc.dram_tensor("mu", [BLOC, NT], f32, kind="ExternalOutput").ap()
    sg_d = nc.dram_tensor("sg", [BLOC, NT], f32, kind="ExternalOutput").ap()
    dbg = bool(int(os.environ.get("BASS_KERNEL_DEBUG", "0")))
    if dbg:
        dbg_d = nc.dram_tensor("dbgyg", [BLOC, 2, G], f32,
                               kind="ExternalOutput").ap()

    woff = []
    off = 0
    for (_ci, co) in CONV:
        woff.append(off)
        off += 5 * co

    trace_sim = bool(int(os.environ.get("BASS_KERNEL_TRACE_SIM", "0")))
    with tile.TileContext(nc, trace_sim=trace_sim) as tc, ExitStack() as ctx:
        singles = ctx.enter_context(tc.tile_pool(name="singles", bufs=1))
        p1p = ctx.enter_context(tc.tile_pool(name="p1p", bufs=2))
        e1p = ctx.enter_context(tc.tile_pool(name="e1p", bufs=2))
        p2p = ctx.enter_context(tc.tile_pool(name="p2p", bufs=2))
        e2p = ctx.enter_context(tc.tile_pool(name="e2p", bufs=2))
        xtbp = ctx.enter_context(tc.tile_pool(name="xtbp", bufs=2))
        cinp = ctx.enter_context(tc.tile_pool(name="cinp", bufs=2))
        ygp = ctx.enter_context(tc.tile_pool(name="ygp", bufs=2))
        ygtp = ctx.enter_context(tc.tile_pool(name="ygtp", bufs=2))
        phip = ctx.enter_context(tc.tile_pool(name="phip", bufs=2))
        ysp = ctx.enter_context(tc.tile_pool(name="ysp", bufs=2))
        smalls = ctx.enter_context(tc.tile_pool(name="smalls", bufs=3))
        psH = ctx.enter_context(tc.tile_pool(name="psH", bufs=1, space="PSUM"))
        psC = ctx.enter_context(tc.tile_pool(name="psC", bufs=1, space="PSUM"))
        psY = ctx.enter_context(tc.tile_pool(name="psY", bufs=1, space="PSUM"))

        # ---- constants into SBUF -------------------------------------------
        tb = singles.tile([128, G], f32)                       # t broadcast on parts
        nc.sync.dma_start(out=tb, in_=tg_d.partition_broadcast(128))
        xs_sb = singles.tile([128, BLOC * KT * 2], f32)
        nc.sync.dma_start(out=xs_sb, in_=xs_d)
        cg2_sb = singles.tile([2, G], f32)
        nc.sync.dma_start(out=cg2_sb, in_=cg2_d)
        ts_sb = singles.tile([128, GT * 2], f32)
        nc.sync.dma_start(out=ts_sb, in_=ts_d)
        wc_sb = singles.tile([32, 330], f32r)
        nc.sync.dma_start(out=wc_sb, in_=wc_d)
        bc_sb = singles.tile([32, 5], f32)
        nc.sync.dma_start(out=bc_sb, in_=bc_d)
        id_sb = singles.tile([2, 2], f32)
        nc.sync.dma_start(out=id_sb, in_=id_d)

        cg2_v = cg2_sb.rearrange("p (c x) -> p c x", x=500)    # [2,2,500]

        state = {}      # per-batch live tiles passed from X(b) to Y(b)

        def emit_X(b):
            xtb = xtbp.tile([128, NT], f32)
            nc.sync.dma_start(out=xtb, in_=xt_d[b].partition_broadcast(128))
            phi = phip.tile([128, 2 * KT], f32r)
            nc.sync.dma_start(out=phi, in_=ph_d[b])
            phi3 = phi.rearrange("p (i c) -> p i c", c=2)

            # stage A: context -> grid (h accumulates over KT k-tiles)
            hps = psH.tile([2, 1024], f32)
            for q in range(KT // 4):
                p1 = p1p.tile([128, 4000], f32)
                for j in range(4):
                    kt = q * 4 + j
                    col = (b * KT + kt) * 2
                    nc.gpsimd.tensor_scalar(
                        out=p1[:, j * 1000:(j + 1) * 1000], in0=tb,
                        scalar1=xs_sb[:, col:col + 1],
                        scalar2=xs_sb[:, col + 1:col + 2],
                        op0=OP.mult, op1=OP.add)
                e1 = e1p.tile([128, 4000], f32r)
                nc.scalar.activation(e1, p1, AF.Exp)
                for j in range(4):
                    kt = q * 4 + j
                    for h in range(2):
                        nc.tensor.matmul(
                            hps[0:2, h * 512:h * 512 + 500],
                            lhsT=(phi3[:, kt, :]),
                            rhs=(e1[:, j * 1000 + h * 500: j * 1000 + h * 500 + 500]),
                            start=(kt == 0), stop=(kt == KT - 1))

            # stage-E weight tiles (exp of target kernel), paired gt per ACT op
            e2s = []
            for pr in range(GT // 2):
                p2 = p2p.tile([128, 2 * NT], f32)
                for j in range(2):
                    gt = pr * 2 + j
                    nc.vector.tensor_scalar(
                        out=p2[:, j * NT:(j + 1) * NT], in0=xtb,
                        scalar1=ts_sb[:, 2 * gt: 2 * gt + 1],
                        scalar2=ts_sb[:, 2 * gt + 1: 2 * gt + 2],
                        op0=OP.mult, op1=OP.add)
                e2 = e2p.tile([128, 2 * NT], f32r)
                nc.scalar.activation(e2, p2, AF.Exp)
                e2s.append(e2)
            state[b] = (hps, e2s)

        def emit_Y(b):
            hps, e2s = state.pop(b)
            # ---- normalize density ----------------------------------------
            cin = cinp.tile([32, 1004], f32r, tag="cin")
            nc.gpsimd.memset(cin[0:2, 0:2].bitcast(f32), 0.0)
            nc.gpsimd.memset(cin[0:2, 1002:1004].bitcast(f32), 0.0)
            hps_v = hps.rearrange("p (c x) -> p c x", x=512)[:, :, 0:500]
            cin_v = cin[0:2, 2:1002].rearrange("p (c x) -> p c x", x=500)
            nc.vector.tensor_tensor(out=cin_v, in0=hps_v, in1=cg2_v, op=OP.mult)
            den = smalls.tile([1, G], f32, tag="sc")
            nc.vector.tensor_scalar(out=den, in0=cin[0:1, 2:1002],
                                    scalar1=1e-8, scalar2=None, op0=OP.add)
            rc = smalls.tile([1, G], f32, tag="sc")
            nc.vector.reciprocal(rc, den)                      # 1/(h0+eps)
            rc2 = smalls.tile([2, G], f32, tag="rc2")
            nc.gpsimd.memset(rc2[0:1, :], 1.0)
            nc.sync.dma_start(out=rc2[1:2, :], in_=rc)
            nc.vector.tensor_tensor(out=cin[0:2, 2:1002], in0=cin[0:2, 2:1002],
                                    in1=rc2, op=OP.mult)

            # ---- CNN --------------------------------------------------------
            cur = cin
            for l, (ci, co) in enumerate(CONV[:3]):
                cps = psC.tile([32, 1024], f32, tag="c")
                for k in range(5):
                    for h in range(2):
                        nc.tensor.matmul(
                            cps[0:co, h * 512:h * 512 + 500],
                            lhsT=(wc_sb[0:ci, woff[l] + k * co: woff[l] + (k + 1) * co]),
                            rhs=(cur[0:ci, k + h * 500: k + h * 500 + 500]),
                            start=(k == 0), stop=(k == 4))
                cps_v = cps.rearrange("p (c x) -> p c x", x=512)[0:co, :, 0:500]
                nxt = cinp.tile([32, 1004], f32r, tag="cin")
                nc.gpsimd.memset(nxt[0:co, 0:2].bitcast(f32), 0.0)
                nc.gpsimd.memset(nxt[0:co, 1002:1004].bitcast(f32), 0.0)
                nxt_v = nxt[0:co, 2:1002].rearrange("p (c x) -> p c x", x=500)
                nc.vector.tensor_scalar(
                    out=nxt_v, in0=cps_v,
                    scalar1=bc_sb[0:co, l:l + 1], scalar2=0.0,
                    op0=OP.add, op1=OP.max)                    # relu(x + bias)
                cur = nxt

            # layer 4 (16 -> 2): one M=1 matmul group per output channel so
            # both land at partition 0 (32-aligned partition offsets only)
            yg = ygp.tile([2, G], f32)
            for co in range(2):
                cp = psC.tile([1, 1024], f32, tag="c")
                for k in range(5):
                    for h in range(2):
                        nc.tensor.matmul(
                            cp[0:1, h * 512:h * 512 + 500],
                            lhsT=(wc_sb[0:16, woff[3] + k * 2 + co:
                                         woff[3] + k * 2 + co + 1]),
                            rhs=(cur[0:16, k + h * 500: k + h * 500 + 500]),
                            start=(k == 0), stop=(k == 4))
                cp_v = cp.rearrange("p (c x) -> p c x", x=512)[0:1, :, 0:500]
                if co == 0:
                    yg_v0 = yg[0:1, :].rearrange("p (c x) -> p c x", x=500)
                    nc.vector.tensor_scalar(out=yg_v0, in0=cp_v,
                                            scalar1=bc_sb[0:1, 3:4],
                                            scalar2=None, op0=OP.add)  # mu_grid
                else:
                    # stable softplus: max(z,0) + ln(1 + exp(-|z|))
                    zb = smalls.tile([1, G], f32, tag="rc2")
                    nc.vector.tensor_scalar(out=zb, in0=cp_v,
                                            scalar1=bc_sb[0:1, 4:5],
                                            scalar2=None, op0=OP.add)
                    na = smalls.tile([1, G], f32, tag="sc")
                    nc.vector.tensor_scalar(out=na.bitcast(mybir.dt.uint32),
                                            in0=zb.bitcast(mybir.dt.uint32),
                                            scalar1=0x80000000, scalar2=None,
                                            op0=OP.bitwise_or)         # -|z|
                    sp = smalls.tile([1, G], f32, tag="sc")
                    nc.scalar.activation(sp, na, AF.Exp)
                    sp2 = smalls.tile([1, G], f32, tag="sc")
                    nc.scalar.activation(sp2, sp, AF.Ln, bias=1.0)
                    rz = smalls.tile([1, G], f32, tag="sc")
                    nc.vector.tensor_scalar(out=rz, in0=zb, scalar1=0.0,
                                            scalar2=None, op0=OP.max)
                    spf = smalls.tile([1, G], f32, tag="sc")
                    nc.vector.tensor_tensor(out=spf, in0=sp2, in1=rz, op=OP.add)
                    nc.sync.dma_start(out=yg[1:2, :], in_=spf)
            if dbg:
                nc.sync.dma_start(out=dbg_d[b], in_=yg)

            # ---- transpose y_grid -> [g, c] tiles ---------------------------
            ygt = ygtp.tile([128, 2 * GT], f32r)
            nc.gpsimd.memset(ygt.bitcast(f32), 0.0)
            for gt in range(GT):
                colw = min(128, G - gt * 128)
                tps = psC.tile([128, 2], f32, tag="c")
                nc.tensor.transpose(tps[0:colw, 0:2],
                                    yg[0:2, gt * 128: gt * 128 + colw],
                                    id_sb)
                nc.vector.tensor_copy(ygt[0:colw, gt * 2: gt * 2 + 2],
                                      tps[0:colw, 0:2])

            # ---- stage C matmuls: grid -> targets ---------------------------
            yps = psY.tile([2, NT], f32)
            for pr in range(GT // 2):
                e2 = e2s[pr]
                for j in range(2):
                    gt = pr * 2 + j
                    for c in range(4):
                        nc.tensor.matmul(
                            yps[0:2, c * 512:(c + 1) * 512],
                            lhsT=(ygt[:, 2 * gt: 2 * gt + 2]),
                            rhs=(e2[:, j * NT + c * 512: j * NT + (c + 1) * 512]),
                            start=(gt == 0), stop=(gt == GT - 1))
            ys = ysp.tile([2, NT], f32)
            nc.vector.tensor_copy(ys, yps)
            nc.sync.dma_start(out=mu_d[b:b + 1, :], in_=ys[0:1, :])
            nc.sync.dma_start(out=sg_d[b:b + 1, :], in_=ys[1:2, :])

        emit_X(0)
        emit_X(1)
        for b in range(BLOC):
            emit_Y(b)
            if b + 2 < BLOC:
                emit_X(b + 2)

    nc.compile()
    return nc


def _host_inputs(x_context, y_context, x_target, ls_x, ls_rho,
                 W1, b1, W2, b2, W3, b3, W4, b4):
    """Per-core input dicts. All heavy math stays on device; these are O(B*N)."""
    t = np.linspace(-2.2, 2.2, G).astype(np.float32).astype(np.float64)
    inv_x = 1.0 / (2.0 * float(ls_x) ** 2)
    inv_r = 1.0 / (2.0 * float(ls_rho) ** 2)

    tg = t.astype(np.float32)
    cg = np.exp(-inv_x * t * t)
    cg2 = np.broadcast_to(cg, (2, G)).astype(np.float32).copy()

    tpad = np.zeros(GPAD, np.float64)
    tpad[:G] = t
    ts = np.zeros((128, GT, 2), np.float64)
    ts[:, :, 0] = (2.0 * inv_r * tpad).reshape(GT, 128).T
    ts[:, :, 1] = (-inv_r * tpad * tpad).reshape(GT, 128).T
    ts[:, GT - 1, :][G - (GT - 1) * 128:, :] = 0.0         # pad rows -> exp(0)=1
    ts = ts.reshape(128, GT * 2).astype(np.float32)

    wc = np.zeros((32, 330), np.float64)
    woff = 0
    for (Wl, (ci, co)) in zip((W1, W2, W3, W4), CONV):
        for k in range(5):
            wc[0:ci, woff + k * co: woff + (k + 1) * co] = np.asarray(Wl)[:, :, k].T
        woff += 5 * co
    wc = wc.astype(np.float32)
    bc = np.zeros((32, 5), np.float64)
    for l, bl in enumerate((b1, b2, b3)):
        bc[0:len(np.asarray(bl)), l] = np.asarray(bl)
    bc[0, 3] = np.asarray(b4)[0]      # L4 mu bias at partition 0
    bc[0, 4] = np.asarray(b4)[1]      # L4 sigma bias at partition 0
    bc = bc.astype(np.float32)
    ident = np.eye(2, dtype=np.float32)

    xc = np.asarray(x_context, np.float64)[:, 0, :]         # [B, NCTX]
    yc = np.asarray(y_context, np.float32)[:, 0, :]
    xt = np.asarray(x_target, np.float64)[:, 0, :]          # [B, NT]
    c2_full = np.exp(-inv_r * xt * xt).astype(np.float32)   # [B, NT] host factor

    # phi lhsT, interleaved [128, (kt, c)]: c=0 -> ones, c=1 -> y_c
    ph_all = np.zeros((B, 128, 2 * KT), np.float32)
    ph_all[:, :, 0::2] = 1.0
    ph_all[:, :, 1::2] = yc.reshape(B, KT, 128).transpose(0, 2, 1)

    in_maps = []
    for c in range(NCORES):
        sl = slice(c * BLOC, (c + 1) * BLOC)
        x = xc[sl].reshape(BLOC, KT, 128)
        xs = np.stack([2.0 * inv_x * x, -inv_x * x * x], axis=-1)  # [b,kt,p,2]
        xs = xs.transpose(2, 0, 1, 3).reshape(128, BLOC * KT * 2).astype(np.float32)
        in_maps.append({
            "tg": tg, "xs": np.ascontiguousarray(xs), "cg2": cg2,
            "ph": np.ascontiguousarray(ph_all[sl]), "ts": ts,
            "xt": xt[sl].astype(np.float32),
            "wc": wc, "bc": bc, "ident": ident,
        })
    return in_maps, c2_full


last_results = None     # BassKernelResults of the most recent run (for test.py)


def kernel(x_context, y_context, x_target, y_target,
           ls_x, ls_rho, W1, b1, W2, b2, W3, b3, W4, b4):
    from concourse.bass_utils import run_bass_kernel_spmd
    global last_results

    assert x_context.shape == (B, 1, NCTX), x_context.shape
    if "nc" not in _cache:
        _cache["nc"] = _build()
    nc = _cache["nc"]

    in_maps, c2_full = _host_inputs(x_context, y_context, x_target, ls_x, ls_rho,
                                    W1, b1, W2, b2, W3, b3, W4, b4)
    trace = bool(int(os.environ.get("BASS_KERNEL_TRACE", "0")))
    res = run_bass_kernel_spmd(nc, in_maps, core_ids=list(range(NCORES)),
                               trace=trace)
    last_results = res

    mu = np.concatenate([r["mu"] for r in res.results], axis=0)    # [B, NT]
    sg = np.concatenate([r["sg"] for r in res.results], axis=0)
    mu = mu * c2_full          # fold the hoisted per-target kernel factor
    sg = sg * c2_full

    # loss on host (float32, mirroring the reference formula)
    yt = np.asarray(y_target, np.float32)[:, 0, :]
    z = (yt - mu) / sg
    logp = -0.5 * z * z - np.log(sg) - np.float32(0.5 * LOG_2PI)
    loss = -np.sum(np.mean(logp, axis=-1))

    mu_o = mu[:, :, None].astype(np.float32)
    sg_o = sg[:, :, None].astype(np.float32)
    return mu_o, sg_o, np.float32(loss)
